# revision 5
# baseline (speedup 1.0000x reference)
"""Multi-head causal attention on 8 Trainium2 NeuronCores (Bass/Tile) — v2.

Sharding: core c -> batch c//4, heads 4*(c%4) .. 4*(c%4)+4  (data + head parallel).
Each core computes its 4 heads' attention plus its partial output projection;
the host sums the 4 partials per batch and adds the output bias.

v2 vs v1:
  - all matmul operands bf16 (x, Wq/Wk/Wv/Wo, Q^T/K^T/AO^T, P^T): same PE
    rate as fp32r (1 row/cycle) but half the DMA + SBUF footprint.
  - softmax denominator broadcast moved off PE: DVE reciprocal of the l-row,
    then gpsimd partition_broadcast + tensor_mul on the Pool engine.
  - head-0 scores, pair-1 Q/K projections and V k-chunks are interleaved so
    the exp pipeline is warm when the first PV matmul issues.
  - output projection is delayed one q-group behind the last head's PV so PE
    never waits on the normalize chain; ost copies go to Pool/DVE, not Act.
  - V bias adds merged into one [128,256] instr per k-chunk (VA4 layout).
"""
from contextlib import ExitStack

import numpy as np

import concourse.bass as bass  # noqa: F401  (bass types via bacc)
import concourse.mybir as mybir
import concourse.tile as tile
from concourse import bacc

S = 2048          # sequence length
DM = 1024         # d_model
DK = 64           # head dim
NCORES = 8
HLOC = 4          # heads per core
CLOC = HLOC * DK  # 256 local channels
NKC = S // 128    # 16 k-chunks
NG = S // 512     # 4 q-groups
NA = DM // 128    # 8 contraction chunks

F32 = mybir.dt.float32
BF16 = mybir.dt.bfloat16
NP_BF16 = mybir.dt.np(BF16)

_prog_cache: dict[tuple, object] = {}

# S-chunk emission schedule (causal): S(h+1) kc-groups per q-group slot of
# head h; S(3) additionally extends into head 3's first slots (kc >= 4*(g+1)
# only, to stay ahead of PV(3,g+1)).
SPLITS_STD = [[12, 13, 14, 15, 0], [1, 2, 8], [3, 4, 9, 10], [5, 6, 7, 11]]
SPLITS_PRE3 = [[12, 13, 14, 15, 0], [1, 2, 8], [3, 4, 9, 10], [5, 11]]
SPLITS_H3 = [[6, 7], [], [], []]


def _pt_offsets(causal: bool) -> tuple[list[int], int]:
    """Start offset of each k-chunk's block inside the packed P^T tile."""
    offs, acc = [], 0
    for kc in range(NKC):
        offs.append(acc)
        acc += (S - 128 * kc) if causal else S
    return offs, acc


def build_program(variant: str, n_iters: int = 1):
    """variant: 'causal' | 'full' | 'generic' (generic = additive mask from DRAM)."""
    causal = variant == "causal"
    generic = variant == "generic"
    nc = bacc.Bacc()

    xT = nc.dram_tensor("xT", [DM, S], BF16, kind="ExternalInput")
    wqT = nc.dram_tensor("wqT", [DM, CLOC], BF16, kind="ExternalInput")
    wkT = nc.dram_tensor("wkT", [DM, CLOC], BF16, kind="ExternalInput")
    wvT = nc.dram_tensor("wvT", [DM, CLOC], BF16, kind="ExternalInput")
    bql = nc.dram_tensor("bql", [CLOC], F32, kind="ExternalInput")
    bkl = nc.dram_tensor("bkl", [CLOC], F32, kind="ExternalInput")
    bvl = nc.dram_tensor("bvl", [CLOC], F32, kind="ExternalInput")
    woT = nc.dram_tensor("woT", [CLOC, DM], BF16, kind="ExternalInput")
    maskT = (
        nc.dram_tensor("maskT", [S, S], F32, kind="ExternalInput") if generic else None
    )
    out_p = nc.dram_tensor("out_p", [S, DM], BF16, kind="ExternalOutput")

    offs, ptw = _pt_offsets(causal)
    Exp = mybir.ActivationFunctionType.Exp

    with tile.TileContext(nc) as tc, ExitStack() as top:
        const = top.enter_context(tc.tile_pool(name="const", bufs=1))
        persist = top.enter_context(tc.tile_pool(name="persist", bufs=1))

        ones_t = const.tile([1, 128], F32, tag="ones")
        nc.gpsimd.memset(ones_t[:], 1.0)
        tri = const.tile([128, 128], BF16, tag="tri")
        nc.gpsimd.memset(tri[:], 1.0)
        nc.gpsimd.affine_select(
            out=tri[:], in_=tri[:], compare_op=mybir.AluOpType.is_ge,
            fill=0.0, base=0, pattern=[[1, 128]], channel_multiplier=-1,
        )
        bvb = const.tile([128, CLOC], F32, tag="bvb")
        bv_row = const.tile([1, CLOC], F32, tag="bvrow")

        woT_t = persist.tile([128, 2, DM], BF16, tag="wo")
        QT = [persist.tile([128, S], BF16, tag=f"qt{j}", name=f"qt{j}") for j in range(2)]
        KT = [persist.tile([128, S], BF16, tag=f"kt{j}", name=f"kt{j}") for j in range(2)]
        AOT = [persist.tile([128, S], BF16, tag=f"aot{j}", name=f"aot{j}") for j in range(2)]
        VA4 = persist.tile([128, NKC, HLOC, DK + 1], BF16, tag="va4", name="va4")
        xT_t = persist.tile([128, NA, S], BF16, tag="xT")
        w_ts = {
            nm: persist.tile([128, NA, CLOC], BF16, tag=f"w{nm}", name=f"w{nm}")
            for nm in ("q", "k", "v")
        }
        b_ts = {
            nm: persist.tile([128, 2], F32, tag=f"b{nm}", name=f"b{nm}")
            for nm in ("q", "k")
        }

        for _it in range(n_iters):
            with ExitStack() as it_ctx:
                psA = it_ctx.enter_context(tc.tile_pool(name="psA", bufs=2, space="PSUM"))
                psS = it_ctx.enter_context(tc.tile_pool(name="psS", bufs=3, space="PSUM"))
                ptp = it_ctx.enter_context(tc.tile_pool(name="ptp", bufs=2 if causal else 1))
                smp = it_ctx.enter_context(tc.tile_pool(name="smp", bufs=2))
                mpool = (
                    it_ctx.enter_context(tc.tile_pool(name="mpool", bufs=3)) if generic else None
                )
                ostp = it_ctx.enter_context(tc.tile_pool(name="ostp", bufs=4))

                # ---------------- DMA issue order (matches PE consumption) --
                xr = xT.rearrange("(a p) s -> p a s", p=128)
                wr = {nm: w.rearrange("(a p) c -> p a c", p=128)
                      for nm, w in (("q", wqT), ("k", wkT), ("v", wvT))}
                nc.sync.dma_start(bv_row[:], bvl[None, :])
                nc.sync.dma_start(w_ts["v"][:, 0:4, :], wr["v"][:, 0:4, :])
                nc.sync.dma_start(xT_t[:, 0:4, 0:128], xr[:, 0:4, 0:128])
                nc.sync.dma_start(w_ts["v"][:, 4:8, :], wr["v"][:, 4:8, :])
                nc.sync.dma_start(xT_t[:, 4:8, 0:128], xr[:, 4:8, 0:128])
                nc.sync.dma_start(xT_t[:, :, 128:512], xr[:, :, 128:512])
                nc.sync.dma_start(w_ts["q"][:], wr["q"][:])
                for nm, bdram in (("q", bql), ("k", bkl)):
                    nc.sync.dma_start(b_ts[nm][:], bdram.rearrange("(a p) -> p a", p=128))
                nc.sync.dma_start(w_ts["k"][:], wr["k"][:])
                for n in range(1, NG):
                    nc.sync.dma_start(
                        xT_t[:, :, 512 * n: 512 * (n + 1)],
                        xr[:, :, 512 * n: 512 * (n + 1)],
                    )
                nc.sync.dma_start(
                    woT_t[:], woT.rearrange("(a p) o -> p a o", p=128)
                )

                # bv broadcast across partitions (also warms up the PE)
                bvp = psA.tile([128, 512], F32, tag="pa", name="bv_ps")
                nc.tensor.matmul(bvp[:, 0:CLOC], ones_t[:], bv_row[:], start=True, stop=True)
                nc.vector.tensor_copy(bvb[:], bvp[:, 0:CLOC])
                nc.gpsimd.memset(VA4[:, :, :, DK: DK + 1], 1.0)

                def emit_V2(kcp):
                    # two k-chunks share one PSUM bank as independent halves
                    vp = psA.tile([128, 512], F32, tag="pa", name="v_ps")
                    for half in range(2):
                        kc = 2 * kcp + half
                        ksl = slice(kc * 128, (kc + 1) * 128)
                        csl = slice(half * CLOC, (half + 1) * CLOC)
                        for a in range(NA):
                            nc.tensor.matmul(
                                vp[:, csl],
                                xT_t[:, a, ksl],
                                w_ts["v"][:, a, :],
                                start=(a == 0),
                                stop=(a == NA - 1),
                            )
                    for half in range(2):
                        kc = 2 * kcp + half
                        csl = slice(half * CLOC, (half + 1) * CLOC)
                        nc.vector.tensor_add(VA4[:, kc, :, 0:DK], vp[:, csl], bvb[:])

                def emit_QK(pair, n, nm):
                    dst = QT[pair] if nm == "q" else KT[pair]
                    qs = slice(512 * n, 512 * (n + 1))
                    ps = psA.tile([128, 512], F32, tag="pa", name="qk_ps")
                    for a in range(NA):
                        nc.tensor.matmul(
                            ps[:],
                            w_ts[nm][:, a, pair * 128: (pair + 1) * 128],
                            xT_t[:, a, qs],
                            start=(a == 0),
                            stop=(a == NA - 1),
                        )
                    nc.vector.tensor_scalar_add(
                        dst[:, qs], ps[:], b_ts[nm][:, pair: pair + 1]
                    )

                PTs = [None] * HLOC

                def emit_S(h, kcs):
                    pair, poff = h // 2, (h % 2) * DK
                    if PTs[h] is None:
                        PTs[h] = ptp.tile([128, ptw], BF16, tag="pt", name=f"pt{h}")
                    PT = PTs[h]
                    for kc in kcs:
                        q0 = kc * 128 if causal else 0
                        ksl = slice(kc * 128, (kc + 1) * 128)
                        qlist = list(range(q0, S, 512))
                        i = 0
                        while i < len(qlist):
                            # fuse two 512-wide score chunks into one 2-bank
                            # PSUM tile so a single exp covers both
                            take = 2 if i + 1 < len(qlist) else 1
                            ps = psS.tile([128, 1024], F32, tag="s", name="s_ps")
                            tot = 0
                            for t in range(take):
                                qs = qlist[i + t]
                                w = min(512, S - qs)
                                nc.tensor.matmul(
                                    ps[:, t * 512: t * 512 + w],
                                    KT[pair][poff: poff + DK, ksl],
                                    QT[pair][poff: poff + DK, qs: qs + w],
                                    start=True,
                                    stop=True,
                                )
                                if generic:
                                    mt = mpool.tile([128, 512], F32, tag="m", name="m_t")
                                    nc.sync.dma_start(mt[:, :w], maskT[ksl, qs: qs + w])
                                    nc.vector.tensor_add(
                                        ps[:, t * 512: t * 512 + w],
                                        ps[:, t * 512: t * 512 + w],
                                        mt[:, :w],
                                    )
                                tot = t * 512 + w
                            po = offs[kc] + qlist[i] - q0
                            nc.scalar.activation(PT[:, po: po + tot], ps[:, :tot], Exp)
                            i += take
                        if causal:
                            # zero strictly-below-diagonal of the boundary tile
                            nc.vector.tensor_mul(
                                PT[:, offs[kc]: offs[kc] + 128],
                                PT[:, offs[kc]: offs[kc] + 128],
                                tri[:],
                            )

                def emit_PV(h, g, interleave_cb=None):
                    pair, poff = h // 2, (h % 2) * DK
                    PT = PTs[h]
                    gs = g * 512
                    aot_full = psA.tile([128, 512], F32, tag="pa", name="ao_ps")
                    ao = aot_full[0: DK + 1, :]
                    kcs = [
                        kc for kc in range(NKC) if (not causal) or kc * 128 < (g + 1) * 512
                    ]
                    for i, kc in enumerate(kcs):
                        if interleave_cb is not None and i % 2 == 1:
                            interleave_cb()
                        q0 = kc * 128 if causal else 0
                        st, sp = (i == 0), (i == len(kcs) - 1)
                        if causal and kc * 128 > gs:
                            d0 = kc * 128 - gs
                            nc.tensor.matmul(
                                ao[:, d0:512],
                                VA4[:, kc, h, :],
                                PT[:, offs[kc]: offs[kc] + 512 - d0],
                                start=st,
                                stop=sp,
                            )
                        else:
                            nc.tensor.matmul(
                                ao[:],
                                VA4[:, kc, h, :],
                                PT[:, offs[kc] + gs - q0: offs[kc] + gs - q0 + 512],
                                start=st,
                                stop=sp,
                            )
                    # normalize: rec = 1/l on DVE, broadcast + multiply on Pool
                    rec = smp.tile([1, 512], F32, tag="rec", name="rec_t")
                    nc.vector.reciprocal(rec[:], ao[DK: DK + 1, :])
                    recb = smp.tile([DK, 512], F32, tag="recb", name="recb_t")
                    nc.gpsimd.partition_broadcast(recb[:], rec[:], channels=DK)
                    nc.vector.tensor_mul(
                        AOT[pair][poff: poff + DK, gs: gs + 512],
                        ao[0:DK, :],
                        recb[:],
                    )

                def emit_oproj(qc):
                    qsl = slice(qc * 128, (qc + 1) * 128)
                    ost = ostp.tile([128, DM], BF16, tag="ost", name="ost_t")
                    ps = psS.tile([128, 1024], F32, tag="s", name="op_ps")
                    for oh in range(2):
                        osl = slice(oh * 512, (oh + 1) * 512)
                        nc.tensor.matmul(
                            ps[:, osl], AOT[0][:, qsl], woT_t[:, 0, osl],
                            start=True, stop=False,
                        )
                        nc.tensor.matmul(
                            ps[:, osl], AOT[1][:, qsl], woT_t[:, 1, osl],
                            start=False, stop=True,
                        )
                    nc.vector.tensor_copy(ost[:, 0:512], ps[:, 0:512])
                    nc.scalar.activation(
                        ost[:, 512:1024], ps[:, 512:1024],
                        mybir.ActivationFunctionType.Copy,
                    )
                    # alternate the out-DMA issue queue: SP drains in the
                    # background, Act is idle during the endgame
                    if qc % 2 == 0:
                        nc.sync.dma_start(out_p[qsl, :], ost[:])
                    else:
                        nc.scalar.dma_start(out_p[qsl, :], ost[:])

                # -------- phase A with head-0 scores interleaved ------------
                for kcp in range(2):
                    emit_V2(kcp)
                for n in range(NG):
                    for nm in ("q", "k"):
                        emit_QK(0, n, nm)
                qk1 = [(n, nm) for n in range(NG) for nm in ("q", "k")]
                for j, kcp in enumerate(range(2, NKC // 2)):
                    emit_V2(kcp)
                    for n, nm in qk1[4 * j // 3: 4 * (j + 1) // 3]:
                        emit_QK(1, n, nm)
                    emit_S(0, [2 * j, 2 * j + 1])
                for kc in range(NKC - 4, NKC):
                    emit_S(0, [kc])

                # -------- attention pipeline --------------------------------
                # S-chunk emission balanced against the exp drain rate
                # (12/10/10/8 chunks per q-group for causal)
                if causal:
                    s_splits = SPLITS_STD
                    s_splits_pre3 = SPLITS_PRE3
                    s_splits_h3 = SPLITS_H3
                else:
                    s_splits = [list(range(4 * g, 4 * g + 4)) for g in range(NG)]
                    s_splits_pre3 = s_splits
                    s_splits_h3 = [[], [], [], []]
                for h in range(HLOC):
                    for g in range(NG):
                        emit_PV(h, g)
                        if h + 1 < HLOC:
                            emit_S(h + 1, s_splits_pre3[g] if h == 2 else s_splits[g])
                        else:
                            emit_S(3, s_splits_h3[g])
                            if g > 0:
                                for qc in range(4 * (g - 1), 4 * g):
                                    emit_oproj(qc)
                for qc in range(4 * (NG - 1), 4 * NG):
                    emit_oproj(qc)

    nc.finalize()
    return nc


def get_program(variant: str, n_iters: int = 1):
    key = (variant, n_iters)
    if key not in _prog_cache:
        _prog_cache[key] = build_program(variant, n_iters)
    return _prog_cache[key]


def classify_mask(mask: np.ndarray) -> str:
    m = np.asarray(mask).reshape(S, S) != 0
    if np.array_equal(m, np.tril(np.ones((S, S), bool))):
        return "causal"
    if m.all():
        return "full"
    return "generic"


def prep_core_inputs(c, x, mask, Wq, bq, Wk, bk, Wv, bv, variant, Wo):
    b, hq = c // 4, c % 4
    cs = slice(hq * CLOC, (hq + 1) * CLOC)
    bf = lambda a: np.ascontiguousarray(np.asarray(a, dtype=np.float32)).astype(NP_BF16)
    f32 = lambda a: np.ascontiguousarray(np.asarray(a, dtype=np.float32))
    im = {
        "xT": bf(np.asarray(x, np.float32)[b].T),
        "wqT": bf(np.asarray(Wq, np.float32)[cs, :].T * 0.125),
        "wkT": bf(np.asarray(Wk, np.float32)[cs, :].T),
        "wvT": bf(np.asarray(Wv, np.float32)[cs, :].T),
        "bql": f32(np.asarray(bq, np.float32)[cs] * 0.125),
        "bkl": f32(np.asarray(bk, np.float32)[cs]),
        "bvl": f32(np.asarray(bv, np.float32)[cs]),
        "woT": bf(np.asarray(Wo, np.float32)[:, cs].T),
    }
    if variant == "generic":
        m = np.asarray(mask).reshape(S, S)
        im["maskT"] = np.where(m.T != 0, np.float32(0.0), np.float32(-1e9))
    return im


def assemble_output(results, bo):
    bo = np.asarray(bo, np.float32)
    out = np.empty((2, S, DM), np.float32)
    for b in range(2):
        acc = results[4 * b]["out_p"].astype(np.float32)
        for j in range(1, 4):
            acc += results[4 * b + j]["out_p"].astype(np.float32)
        out[b] = acc + bo[None, :]
    return out


def kernel(x, mask, Wq, bq, Wk, bk, Wv, bv, Wo, bo) -> np.ndarray:
    from concourse.bass_utils import run_bass_kernel_spmd

    variant = classify_mask(mask)
    nc = get_program(variant)
    in_maps = [
        prep_core_inputs(c, x, mask, Wq, bq, Wk, bk, Wv, bv, variant, Wo)
        for c in range(NCORES)
    ]
    res = run_bass_kernel_spmd(nc, in_maps, core_ids=list(range(NCORES))).results
    return assemble_output(res, bo)
